# revision 1
# baseline (speedup 1.0000x reference)
"""Trainium2 Bass kernel for nn_Concat_73607149519362.

Math (decomposed concat-MLP attention score):
    score[b, d, e] = dec[b, d] @ w_dec + enc[b, e] @ w_enc + bias

Sharding: data-parallel over batch, 32 batches / 8 cores = 4 per core.
Raw bass with explicit semaphores (the Tile framework's attached
sync_info waits don't encode on this toolchain).

Per-core per-batch pipeline:
  SP  : enc (2x2MB) + dec (2MB) input DMAs, double-buffered slots.
        DRAM views are p-major so every partition reads one contiguous
        16-32KB run (cheap descriptors).
  DVE : one fp32 tensor_tensor multiply per 128-row chunk against the
        broadcast weight row; also the free-axis reduce for the last 3
        dec chunks (engine balance).
  ACT : activation(Copy, accum_out) reduces for the 8 enc chunks + 1 dec
        chunk; the enc_proj row copy (PSUM->SBUF, permuted view, bias
        folded in); the 4 output builds (Identity, per-partition bias =
        dec projection); the output DMA issue.
  PE  : 8 tiny transposes flatten enc_proj columns into a (1, enc) PSUM
        row; 2 ones outer-product matmuls rebroadcast it to (128, enc)
        PSUM for the ACT builds.
  Pool: one-time broadcast loads of weights/bias/identity/ones.
"""

import os
from contextlib import ExitStack

os.environ.setdefault("JAX_PLATFORMS", "axon")

import numpy as np

import concourse.bass as bass
import concourse.mybir as mybir
from concourse.bass_utils import run_bass_kernel_spmd

B, DEC, ENC, DIM = 32, 512, 1024, 1024
NCORES = 8
BPC = B // NCORES  # batches per core

F32 = mybir.dt.float32
P = 128
NSCR = 4  # rotating DVE-product scratch slots


def _build(bpc=BPC, dec=DEC, enc=ENC, dim=DIM):
    nc = bass.Bass("TRN2")
    dec_h = nc.dram_tensor("dec_in", [bpc * dec, dim], F32, kind="ExternalInput")
    enc_h = nc.dram_tensor("enc_in", [bpc * enc, dim], F32, kind="ExternalInput")
    wenc_h = nc.dram_tensor("w_enc", [1, dim], F32, kind="ExternalInput")
    wdec_h = nc.dram_tensor("w_dec", [1, dim], F32, kind="ExternalInput")
    bias_h = nc.dram_tensor("bias", [1, 1], F32, kind="ExternalInput")
    ident_h = nc.dram_tensor("ident_in", [P, P], F32, kind="ExternalInput")
    ones_h = nc.dram_tensor("ones_in", [1, P], F32, kind="ExternalInput")
    out_h = nc.dram_tensor("out", [bpc * dec, enc], F32, kind="ExternalOutput")

    te = enc // P  # enc 128-row chunks per batch
    td = dec // P  # dec 128-row chunks per batch
    assert te % 2 == 0
    nprod = te + td
    ndve_red = min(3, td)  # dec reduces done on DVE (engine balance)
    nact_red = nprod - ndve_red  # reduces done on ACT
    DV = nprod + ndve_red  # DVE s_mult increments per batch
    A = nact_red + 1 + td  # ACT s_acc increments per batch
    nblk = min(512, enc)
    nh = enc // nblk
    eh = te // 2  # enc chunks per half-load

    # p-major views: within a batch, partition p owns rows p*t..p*t+t-1,
    # i.e. one contiguous run per partition per DMA.
    dec_r = dec_h.ap().rearrange("(b p t) d -> b p t d", p=P, t=td)
    enc_r = enc_h.ap().rearrange("(b p t) d -> b p t d", p=P, t=te)
    out_r = out_h.ap().rearrange("(b p t) e -> b p t e", p=P, t=td)

    # DVE op index (1-based s_mult count) for the multiply of chunk k:
    # order: m0..m(nact_red-1), then (mult, reduce) pairs for DVE chunks.
    def mult_count(b, k):
        if k < nact_red:
            return DV * b + k + 1
        return DV * b + nact_red + 2 * (k - nact_red) + 1

    def act_accum_count(b, k):  # ACT s_acc count after accum of chunk k
        return A * b + k + 1

    with ExitStack() as ctx:

        def sb(name, shape):
            return ctx.enter_context(nc.sbuf_tensor(name, shape, F32))

        w_enc_b = sb("w_enc_b", [P, dim])
        w_dec_b = sb("w_dec_b", [P, dim])
        bias_b = sb("bias_b", [P, 1])
        ident = sb("ident", [P, P])
        ones_row = sb("ones_row", [1, P])
        enc_t = [sb(f"enc_t{i}", [P, te, dim]) for i in range(2)]
        dec_t = [sb(f"dec_t{i}", [P, td, dim]) for i in range(2)]
        scr = [sb(f"scr{i}", [P, dim]) for i in range(NSCR)]
        eproj = [sb(f"eproj{i}", [P, te]) for i in range(2)]
        dproj = [sb(f"dproj{i}", [P, td]) for i in range(2)]
        enc_row = [sb(f"enc_row{i}", [1, enc]) for i in range(2)]
        out_t = [sb(f"out_t{i}", [P, td, enc]) for i in range(2)]
        tp_row = ctx.enter_context(nc.psum_tensor("tp_row", [1, enc], F32))
        ebc = ctx.enter_context(nc.psum_tensor("ebc", [P, enc], F32))

        s_gp = ctx.enter_context(nc.semaphore(name="s_gp"))
        # enc load groups: two 1-chunk groups first (fast compute start),
        # then 2-chunk groups
        egrp = [(0, 1), (1, 2)] + [(lo, lo + 2) for lo in range(2, te, 2)]
        nqe = len(egrp)
        nqd = (td + 1) // 2  # dec load groups
        s_enc = [
            [ctx.enter_context(nc.semaphore(name=f"s_enc{i}{h}")) for h in range(nqe)]
            for i in range(2)
        ]
        s_dec = [
            [ctx.enter_context(nc.semaphore(name=f"s_dec{i}{h}")) for h in range(nqd)]
            for i in range(2)
        ]
        s_mult = ctx.enter_context(nc.semaphore(name="s_mult"))
        s_acc = ctx.enter_context(nc.semaphore(name="s_acc"))
        s_pe = ctx.enter_context(nc.semaphore(name="s_pe"))
        s_outdma = [
            ctx.enter_context(nc.semaphore(name=f"s_outdma{i}")) for i in range(2)
        ]

        with nc.Block(no_gpsimd_drain=True) as block:

            @block.sync
            def _(sync):
                # issued DMA completion points, for depth-2 issue pipelining
                issued = []

                def issue(dma_fn, sem, val, war):
                    if war is not None:
                        sync.wait_ge(s_mult, war)
                    if len(issued) >= 3:
                        psem, pval = issued[-3]
                        sync.wait_ge(psem, pval)
                    dma_fn().then_inc(sem, 16)
                    issued.append((sem, val))

                for b in range(bpc):
                    use = 16 * (b // 2 + 1)
                    for q in range(nqe):
                        lo, hi = egrp[q]
                        war = (
                            mult_count(b - 2, hi - 1) if b >= 2 else None
                        )
                        issue(
                            lambda lo=lo, hi=hi, b=b: sync.dma_start(
                                enc_t[b % 2].ap()[:, lo:hi, :],
                                enc_r[b][:, lo:hi, :],
                            ),
                            s_enc[b % 2][q],
                            use,
                            war,
                        )
                    for q in range(nqd):
                        lo, hi = 2 * q, min(2 * q + 2, td)
                        war = (
                            mult_count(b - 2, te + hi - 1) if b >= 2 else None
                        )
                        issue(
                            lambda lo=lo, hi=hi, b=b: sync.dma_start(
                                dec_t[b % 2].ap()[:, lo:hi, :],
                                dec_r[b][:, lo:hi, :],
                            ),
                            s_dec[b % 2][q],
                            use,
                            war,
                        )


            @block.gpsimd
            def _(gpsimd):
                gpsimd.dma_start(
                    w_enc_b.ap(), wenc_h.ap().to_broadcast((P, dim))
                ).then_inc(s_gp, 16)
                gpsimd.dma_start(
                    w_dec_b.ap(), wdec_h.ap().to_broadcast((P, dim))
                ).then_inc(s_gp, 16)
                gpsimd.wait_ge(s_gp, 32)  # settle: make 32 a valid wait point
                gpsimd.dma_start(
                    bias_b.ap(), bias_h.ap().to_broadcast((P, 1))
                ).then_inc(s_gp, 16)
                gpsimd.wait_ge(s_gp, 48)  # settle: make 48 a valid wait point
                gpsimd.dma_start(ident.ap(), ident_h.ap()).then_inc(s_gp, 16)
                gpsimd.dma_start(ones_row.ap(), ones_h.ap()).then_inc(s_gp, 16)
                # ship outputs as their builds finish (ACT stays compute-only)
                for b in range(bpc):
                    if b < bpc - 1:
                        gpsimd.wait_ge(s_acc, A * (b + 1))
                        nc.gpsimd.dma_start(
                            out_r[b], out_t[b % 2].ap()
                        ).then_inc(s_outdma[b % 2], 16)
                    else:
                        # tail: slice the last batch so it drains early
                        for t in range(td):
                            gpsimd.wait_ge(
                                s_acc, A * b + nact_red + 1 + t + 1
                            )
                            nc.gpsimd.dma_start(
                                out_r[b][:, t, :], out_t[b % 2].ap()[:, t, :]
                            ).then_inc(s_outdma[b % 2], 16)

            @block.vector
            def _(vector):
                for b in range(bpc):
                    for k in range(nprod):
                        if b == 0 and k == 0:
                            vector.wait_ge(s_gp, 32)  # weight rows loaded
                        if k < te:
                            for qi, (lo, hi) in enumerate(egrp):
                                if k == lo:
                                    vector.wait_ge(
                                        s_enc[b % 2][qi], 16 * (b // 2 + 1)
                                    )
                        if k >= te and (k - te) % 2 == 0:
                            vector.wait_ge(
                                s_dec[b % 2][(k - te) // 2], 16 * (b // 2 + 1)
                            )
                        g = nprod * b + k  # global mult index -> scratch slot
                        if g >= NSCR and g % 2 == 0:
                            # cover the slots of this mult and the next one
                            need = 0
                            for gg in (g - NSCR, g + 1 - NSCR):
                                if gg >= 0:
                                    b2, k2 = divmod(gg, nprod)
                                    if k2 < nact_red:
                                        need = max(need, act_accum_count(b2, k2))
                            if need:
                                vector.wait_ge(s_acc, need)
                        if k < te:
                            src, wsrc = enc_t[b % 2].ap()[:, k, :], w_enc_b
                        else:
                            src, wsrc = dec_t[b % 2].ap()[:, k - te, :], w_dec_b
                        nc.vector.tensor_tensor(
                            out=scr[g % NSCR].ap(),
                            in0=src,
                            in1=wsrc.ap(),
                            op=mybir.AluOpType.mult,
                        ).then_inc(s_mult, 1)
                        if k >= nact_red:
                            # reduce this dec chunk ourselves (engine balance)
                            if b >= 2 and k == nact_red:
                                # WAR: dproj slot free once b-2's builds read it
                                vector.wait_ge(s_acc, A * (b - 1))
                            # self-wait: our multiply's writes must retire
                            vector.wait_ge(s_mult, mult_count(b, k))
                            nc.vector.tensor_reduce(
                                out=dproj[b % 2].ap()[:, k - te : k - te + 1],
                                in_=scr[g % NSCR].ap(),
                                axis=mybir.AxisListType.X,
                                op=mybir.AluOpType.add,
                            ).then_inc(s_mult, 1)

            @block.scalar
            def _(scalar):
                for b in range(bpc):
                    if b >= 2:
                        # WAR: eproj/dproj slot free once batch b-2's PE used it.
                        scalar.wait_ge(s_pe, 2 * (b - 1))
                    for k in range(nact_red):
                        if k % 2 == 0:
                            scalar.wait_ge(
                                s_mult, mult_count(b, min(k + 1, nact_red - 1))
                            )
                        if k < te:
                            tgt = eproj[b % 2].ap()[:, k : k + 1]
                        else:
                            tgt = dproj[b % 2].ap()[:, k - te : k - te + 1]
                        g = nprod * b + k
                        nc.scalar.activation(
                            out=scr[g % NSCR].ap(),
                            in_=scr[g % NSCR].ap(),
                            func=mybir.ActivationFunctionType.Copy,
                            accum_out=tgt,
                        ).then_inc(s_acc, 1)
                    # enc_proj row: PSUM -> SBUF, permuted to p-major order,
                    # with the mlp bias folded in.
                    if b == 0:
                        scalar.wait_ge(s_gp, 48)
                    scalar.wait_ge(s_pe, 2 * b + 1)
                    nc.scalar.add(
                        enc_row[b % 2].ap().rearrange("o (p t) -> o p t", p=P),
                        tp_row.ap().rearrange("o (t p) -> o p t", p=P),
                        add=bias_b.ap()[0:1, 0:1],
                    ).then_inc(s_acc, 1)
                    # output builds: out = ebc + dec_proj (per-partition bias).
                    scalar.wait_ge(s_pe, 2 * b + 2)
                    if b >= 2:
                        scalar.wait_ge(s_outdma[b % 2], 16 * (b // 2))
                    for t in range(td):
                        k = te + t
                        if k >= nact_red:
                            # this dproj column comes from DVE's reduce
                            scalar.wait_ge(s_mult, mult_count(b, k) + 1)
                        nc.scalar.add(
                            out_t[b % 2].ap()[:, t, :],
                            ebc.ap(),
                            add=dproj[b % 2].ap()[:, t : t + 1],
                        ).then_inc(s_acc, 1)

            @block.tensor
            def _(pe):
                for b in range(bpc):
                    if b == 0:
                        pe.wait_ge(s_gp, 80)  # ident + ones ready
                    pe.wait_ge(s_acc, A * b + te)  # eproj columns ready
                    last = None
                    for t in range(te):
                        last = nc.tensor.transpose(
                            tp_row.ap()[0:1, t * P : (t + 1) * P],
                            eproj[b % 2].ap()[:, t : t + 1],
                            ident.ap(),
                        )
                    last.then_inc(s_pe, 1)
                    pe.wait_ge(s_acc, A * b + nact_red + 1)  # enc_row ready
                    last = None
                    for h in range(nh):
                        last = nc.tensor.matmul(
                            ebc.ap()[:, h * nblk : (h + 1) * nblk],
                            ones_row.ap(),
                            enc_row[b % 2].ap()[0:1, h * nblk : (h + 1) * nblk],
                            start=True,
                            stop=True,
                        )
                    last.then_inc(s_pe, 1)

    return nc


_NC_CACHE = {}


def _get_nc():
    if "nc" not in _NC_CACHE:
        _NC_CACHE["nc"] = _build()
    return _NC_CACHE["nc"]


_IDENT = np.eye(P, dtype=np.float32)
_ONES = np.ones((1, P), dtype=np.float32)


def _shard_inputs(decoder_states, encoder_states, mlp_weight, mlp_bias):
    decoder_states = np.ascontiguousarray(np.asarray(decoder_states, dtype=np.float32))
    encoder_states = np.ascontiguousarray(np.asarray(encoder_states, dtype=np.float32))
    mlp_weight = np.asarray(mlp_weight, dtype=np.float32).reshape(1, 2 * DIM)
    mlp_bias = np.ascontiguousarray(
        np.asarray(mlp_bias, dtype=np.float32).reshape(1, 1)
    )

    w_enc = np.ascontiguousarray(mlp_weight[:, :DIM])
    w_dec = np.ascontiguousarray(mlp_weight[:, DIM:])

    in_maps = []
    for i in range(NCORES):
        lo = i * BPC
        in_maps.append(
            {
                "dec_in": decoder_states[lo : lo + BPC].reshape(BPC * DEC, DIM),
                "enc_in": encoder_states[lo : lo + BPC].reshape(BPC * ENC, DIM),
                "w_enc": w_enc,
                "w_dec": w_dec,
                "bias": mlp_bias,
                "ident_in": _IDENT,
                "ones_in": _ONES,
            }
        )
    return in_maps


def _gather(res):
    shards = [r["out"].reshape(BPC, DEC, ENC) for r in res.results]
    return np.concatenate(shards, axis=0)


def kernel(decoder_states, encoder_states, step, mlp_weight, mlp_bias, **_ignored):
    in_maps = _shard_inputs(decoder_states, encoder_states, mlp_weight, mlp_bias)
    res = run_bass_kernel_spmd(_get_nc(), in_maps, core_ids=list(range(NCORES)))
    return _gather(res)



# revision 6
# speedup vs baseline: 1.5022x; 1.5022x over previous
"""Trainium2 Bass kernel for nn_Concat_73607149519362.

Math (decomposed concat-MLP attention score):
    score[b, d, e] = dec[b, d] @ w_dec + enc[b, e] @ w_enc + bias

Sharding: data-parallel over batch, 32 batches / 8 cores = 4 per core.

v2: fp16 input/output DMA (fp32 accumulation; rel err ~3e-4 vs the 2e-2
gate) halves HBM traffic to ~17.3 MB/core. At fp16 all 4 batches fit in
SBUF at once, so every input DMA is issued eagerly with no WAR hazards.

Per-core pipeline:
  SP  : all enc/dec input DMAs issued up-front (p-major DRAM views ->
        8-16KB contiguous runs per partition).
  Pool: one-time weight/bias/ident/ones loads on parallel semaphores;
        output DMAs as batches complete (last batch sliced per chunk).
  DVE : one fused scalar_tensor_tensor (mult + free-axis sum accum)
        per 128-row chunk -> eproj/dproj columns, fp32.
  PE  : per batch, 8 tiny transposes flatten eproj to a (1, enc) PSUM
        row; 2 ones-outer-product matmuls rebroadcast the bias-folded
        row to (128, enc) PSUM.
  ACT : enc_row = permute(tp_row) + bias; 4 output builds per batch
        (Identity + per-partition dproj bias), f32 PSUM -> fp16 SBUF.
"""

import os
from contextlib import ExitStack

os.environ.setdefault("JAX_PLATFORMS", "axon")

import numpy as np

import concourse.bass as bass
import concourse.mybir as mybir
from concourse.bass_utils import run_bass_kernel_spmd

B, DEC, ENC, DIM = 32, 512, 1024, 1024
NCORES = 8
BPC = B // NCORES  # batches per core

F32 = mybir.dt.float32
F16 = mybir.dt.float16
P = 128


def _build(bpc=BPC, dec=DEC, enc=ENC, dim=DIM):
    nc = bass.Bass("TRN2")
    dec_h = nc.dram_tensor("dec_in", [bpc * dec, dim], F16, kind="ExternalInput")
    enc_h = nc.dram_tensor("enc_in", [bpc * enc, dim], F16, kind="ExternalInput")
    wenc_h = nc.dram_tensor("w_enc", [1, dim], F16, kind="ExternalInput")
    wdec_h = nc.dram_tensor("w_dec", [1, dim], F16, kind="ExternalInput")
    bias_h = nc.dram_tensor("bias", [1, 1], F32, kind="ExternalInput")
    ident_h = nc.dram_tensor("ident_in", [P, P], F32, kind="ExternalInput")
    ones_h = nc.dram_tensor("ones_in", [1, P], F32, kind="ExternalInput")
    out_h = nc.dram_tensor("out", [bpc * dec, enc], F16, kind="ExternalOutput")

    te = enc // P  # enc 128-row chunks per batch
    td = dec // P  # dec 128-row chunks per batch
    nk = te + td  # DVE fused ops per batch
    A = 1 + td  # ACT ops per batch (enc_row + builds)
    nblk = 512  # PSUM-bank-sized matmul block
    nh = enc // nblk

    # p-major views: within a batch, partition p owns rows p*t..p*t+t-1,
    # i.e. one contiguous run per partition per DMA.
    dec_r = dec_h.ap().rearrange("(b p t) d -> b p t d", p=P, t=td)
    enc_r = enc_h.ap().rearrange("(b p t) d -> b p t d", p=P, t=te)
    out_r = out_h.ap().rearrange("(b p t) e -> b p t e", p=P, t=td)

    # input DMA groups (lo, hi) in chunk units; first/last batches split
    # finer to shorten ramp and tail
    def enc_groups(b):
        if b == 0:
            return [(0, 2), (2, 4), (4, te)]
        if b == bpc - 1:
            return [(0, te // 2), (te // 2, te)]
        return [(0, te)]

    def dec_groups(b):
        if b == bpc - 1:
            return [(t, t + 1) for t in range(td)]
        return [(0, td)]

    with ExitStack() as ctx:

        def sb(name, shape, dt=F32):
            return ctx.enter_context(nc.sbuf_tensor(name, shape, dt))

        w_enc_b = sb("w_enc_b", [P, dim], F16)
        w_dec_b = sb("w_dec_b", [P, dim], F16)
        bias_b = sb("bias_b", [P, 1])
        ident = sb("ident", [P, P])
        ones_row = sb("ones_row", [1, P])
        enc_t = [sb(f"enc_t{i}", [P, te, dim], F16) for i in range(bpc)]
        dec_t = [sb(f"dec_t{i}", [P, td, dim], F16) for i in range(bpc)]
        scr = [sb(f"scr{i}", [P, dim], F16) for i in range(2)]
        eproj = [sb(f"eproj{i}", [P, te]) for i in range(bpc)]
        dproj = [sb(f"dproj{i}", [P, td]) for i in range(bpc)]
        enc_row = [sb(f"enc_row{i}", [1, enc]) for i in range(bpc)]
        out_t = [sb(f"out_t{i}", [P, td, enc], F16) for i in range(bpc)]
        tp_row = [
            ctx.enter_context(nc.psum_tensor(f"tp_row{i}", [1, enc], F32))
            for i in range(2)
        ]
        ebc = [
            ctx.enter_context(nc.psum_tensor(f"ebc{i}", [P, enc], F32))
            for i in range(2)
        ]

        s_we = ctx.enter_context(nc.semaphore(name="s_we"))
        s_wd = ctx.enter_context(nc.semaphore(name="s_wd"))
        s_misc = ctx.enter_context(nc.semaphore(name="s_misc"))
        s_enc = [
            [
                ctx.enter_context(nc.semaphore(name=f"s_enc{b}g{g}"))
                for g in range(len(enc_groups(b)))
            ]
            for b in range(bpc)
        ]
        s_dec = [
            [
                ctx.enter_context(nc.semaphore(name=f"s_dec{b}g{g}"))
                for g in range(len(dec_groups(b)))
            ]
            for b in range(bpc)
        ]
        s_mult = ctx.enter_context(nc.semaphore(name="s_mult"))
        s_acc = ctx.enter_context(nc.semaphore(name="s_acc"))
        s_pe = ctx.enter_context(nc.semaphore(name="s_pe"))
        s_out = ctx.enter_context(nc.semaphore(name="s_out"))

        with nc.Block(no_gpsimd_drain=True) as block:

            @block.sync
            def _(sync):
                for b in range(bpc):
                    for g, (lo, hi) in enumerate(enc_groups(b)):
                        sync.dma_start(
                            enc_t[b].ap()[:, lo:hi, :], enc_r[b][:, lo:hi, :]
                        ).then_inc(s_enc[b][g], 16)
                    for g, (lo, hi) in enumerate(dec_groups(b)):
                        sync.dma_start(
                            dec_t[b].ap()[:, lo:hi, :], dec_r[b][:, lo:hi, :]
                        ).then_inc(s_dec[b][g], 16)

            @block.gpsimd
            def _(gpsimd):
                gpsimd.dma_start(
                    w_enc_b.ap(), wenc_h.ap().to_broadcast((P, dim))
                ).then_inc(s_we, 16)
                gpsimd.dma_start(
                    w_dec_b.ap(), wdec_h.ap().to_broadcast((P, dim))
                ).then_inc(s_wd, 16)
                gpsimd.dma_start(
                    bias_b.ap(), bias_h.ap().to_broadcast((P, 1))
                ).then_inc(s_misc, 16)
                gpsimd.dma_start(ident.ap(), ident_h.ap()).then_inc(s_misc, 16)
                gpsimd.dma_start(ones_row.ap(), ones_h.ap()).then_inc(s_misc, 16)
                # ship outputs as their builds finish
                for b in range(bpc):
                    if b < bpc - 1:
                        gpsimd.wait_ge(s_acc, A * (b + 1))
                        nc.gpsimd.dma_start(out_r[b], out_t[b].ap()).then_inc(
                            s_out, 16
                        )
                    else:
                        for t in range(td):
                            gpsimd.wait_ge(s_acc, A * b + 1 + t + 1)
                            nc.gpsimd.dma_start(
                                out_r[b][:, t, :], out_t[b].ap()[:, t, :]
                            ).then_inc(s_out, 16)

            @block.vector
            def _(vector):
                for b in range(bpc):
                    for k in range(nk):
                        if b == 0 and k == 0:
                            vector.wait_ge(s_we, 16)
                        if b == 0 and k == te:
                            vector.wait_ge(s_wd, 16)
                        if k < te:
                            for g, (lo, hi) in enumerate(enc_groups(b)):
                                if k == lo:
                                    vector.wait_ge(s_enc[b][g], 16)
                            src, wsrc = enc_t[b].ap()[:, k, :], w_enc_b
                            tgt = eproj[b].ap()[:, k : k + 1]
                        else:
                            kd = k - te
                            for g, (lo, hi) in enumerate(dec_groups(b)):
                                if kd == lo:
                                    vector.wait_ge(s_dec[b][g], 16)
                            src, wsrc = dec_t[b].ap()[:, kd, :], w_dec_b
                            tgt = dproj[b].ap()[:, kd : kd + 1]
                        nc.vector.scalar_tensor_tensor(
                            out=scr[k % 2].ap(),
                            in0=src,
                            scalar=1.0,
                            in1=wsrc.ap(),
                            op0=mybir.AluOpType.mult,
                            op1=mybir.AluOpType.mult,
                            accum_out=tgt,
                        ).then_inc(s_mult, 1)

            @block.tensor
            def _(pe):
                for b in range(bpc):
                    if b == 0:
                        pe.wait_ge(s_misc, 48)  # ident + ones ready
                    if b >= 2:
                        # tp_row slot free once b-2's enc_row add read it
                        pe.wait_ge(s_acc, A * (b - 2) + 1)
                    pe.wait_ge(s_mult, nk * b + te)  # eproj columns ready
                    last = None
                    for t in range(te):
                        last = nc.tensor.transpose(
                            tp_row[b % 2].ap()[0:1, t * P : (t + 1) * P],
                            eproj[b].ap()[:, t : t + 1],
                            ident.ap(),
                        )
                    last.then_inc(s_pe, 1)
                    pe.wait_ge(s_acc, A * b + 1)  # enc_row ready
                    last = None
                    for h in range(nh):
                        last = nc.tensor.matmul(
                            ebc[b % 2].ap()[:, h * nblk : (h + 1) * nblk],
                            ones_row.ap(),
                            enc_row[b].ap()[0:1, h * nblk : (h + 1) * nblk],
                            start=True,
                            stop=True,
                        )
                    last.then_inc(s_pe, 1)

            @block.scalar
            def _(scalar):
                for b in range(bpc):
                    # enc_proj row: PSUM -> SBUF, permuted to row order,
                    # with the mlp bias folded in.
                    if b == 0:
                        scalar.wait_ge(s_misc, 48)
                    scalar.wait_ge(s_pe, 2 * b + 1)
                    nc.scalar.add(
                        enc_row[b].ap().rearrange("o (p t) -> o p t", p=P),
                        tp_row[b % 2].ap().rearrange("o (t p) -> o p t", p=P),
                        add=bias_b.ap()[0:1, 0:1],
                    ).then_inc(s_acc, 1)
                    # output builds: out = ebc + dec_proj (per-partition bias)
                    scalar.wait_ge(s_pe, 2 * b + 2)
                    for t in range(td):
                        scalar.wait_ge(s_mult, nk * b + te + t + 1)
                        nc.scalar.add(
                            out_t[b].ap()[:, t, :],
                            ebc[b % 2].ap(),
                            add=dproj[b].ap()[:, t : t + 1],
                        ).then_inc(s_acc, 1)

    return nc


_NC_CACHE = {}


def _get_nc():
    if "nc" not in _NC_CACHE:
        _NC_CACHE["nc"] = _build()
    return _NC_CACHE["nc"]


_IDENT = np.eye(P, dtype=np.float32)
_ONES = np.ones((1, P), dtype=np.float32)


def _shard_inputs(decoder_states, encoder_states, mlp_weight, mlp_bias):
    decoder_states = np.asarray(decoder_states, dtype=np.float32).astype(np.float16)
    encoder_states = np.asarray(encoder_states, dtype=np.float32).astype(np.float16)
    decoder_states = np.ascontiguousarray(decoder_states)
    encoder_states = np.ascontiguousarray(encoder_states)
    mlp_weight = np.asarray(mlp_weight, dtype=np.float32).reshape(1, 2 * DIM)
    mlp_bias = np.ascontiguousarray(
        np.asarray(mlp_bias, dtype=np.float32).reshape(1, 1)
    )

    w_enc = np.ascontiguousarray(mlp_weight[:, :DIM].astype(np.float16))
    w_dec = np.ascontiguousarray(mlp_weight[:, DIM:].astype(np.float16))

    in_maps = []
    for i in range(NCORES):
        lo = i * BPC
        in_maps.append(
            {
                "dec_in": decoder_states[lo : lo + BPC].reshape(BPC * DEC, DIM),
                "enc_in": encoder_states[lo : lo + BPC].reshape(BPC * ENC, DIM),
                "w_enc": w_enc,
                "w_dec": w_dec,
                "bias": mlp_bias,
                "ident_in": _IDENT,
                "ones_in": _ONES,
            }
        )
    return in_maps


def _gather(res):
    shards = [
        r["out"].astype(np.float32).reshape(BPC, DEC, ENC) for r in res.results
    ]
    return np.concatenate(shards, axis=0)


def kernel(decoder_states, encoder_states, step, mlp_weight, mlp_bias, **_ignored):
    in_maps = _shard_inputs(decoder_states, encoder_states, mlp_weight, mlp_bias)
    res = run_bass_kernel_spmd(_get_nc(), in_maps, core_ids=list(range(NCORES)))
    return _gather(res)


# revision 12
# speedup vs baseline: 1.5128x; 1.0071x over previous
"""Trainium2 Bass kernel for nn_Concat_73607149519362.

Math (decomposed concat-MLP attention score):
    score[b, d, e] = dec[b, d] @ w_dec + enc[b, e] @ w_enc + bias

Sharding: data-parallel over batch, 32 batches / 8 cores = 4 per core.

fp16 input/output DMA (fp32 accumulation; rel err ~3e-4 vs the 2e-2
gate) halves HBM traffic to ~17.3 MB/core. At fp16 all 4 batches fit in
SBUF at once, so every input DMA is issued eagerly with no WAR hazards.

Per-core pipeline:
  SP  : all enc/dec input DMAs issued up-front (p-major DRAM views ->
        8-16KB contiguous runs per partition).
  Pool: one-time weight/bias/ident/ones loads on parallel semaphores;
        output DMAs as batches complete (last batch sliced per chunk).
  DVE : one fused scalar_tensor_tensor (mult + free-axis sum accum)
        per 128-row chunk -> eproj/dproj columns, fp32.
  PE  : per batch, 8 tiny transposes flatten eproj to a (1, enc) PSUM
        row; 2 ones-outer-product matmuls rebroadcast the bias-folded
        row to (128, enc) PSUM.
  ACT : enc_row = permute(tp_row) + bias; 4 output builds per batch
        (Identity + per-partition dproj bias), f32 PSUM -> fp16 SBUF.
"""

import os
from contextlib import ExitStack

os.environ.setdefault("JAX_PLATFORMS", "axon")

import numpy as np

import concourse.bass as bass
import concourse.mybir as mybir
from concourse.bass_utils import run_bass_kernel_spmd

B, DEC, ENC, DIM = 32, 512, 1024, 1024
NCORES = 8
BPC = B // NCORES  # batches per core

F32 = mybir.dt.float32
F16 = mybir.dt.float16
P = 128


def _build(bpc=BPC, dec=DEC, enc=ENC, dim=DIM):
    nc = bass.Bass("TRN2")
    dec_h = nc.dram_tensor("dec_in", [bpc * dec, dim], F16, kind="ExternalInput")
    enc_h = nc.dram_tensor("enc_in", [bpc * enc, dim], F16, kind="ExternalInput")
    wenc_h = nc.dram_tensor("w_enc", [1, dim], F16, kind="ExternalInput")
    wdec_h = nc.dram_tensor("w_dec", [1, dim], F16, kind="ExternalInput")
    bias_h = nc.dram_tensor("bias", [1, 1], F32, kind="ExternalInput")
    ident_h = nc.dram_tensor("ident_in", [P, P], F32, kind="ExternalInput")
    ones_h = nc.dram_tensor("ones_in", [1, P], F32, kind="ExternalInput")
    out_h = nc.dram_tensor("out", [bpc * dec, enc], F16, kind="ExternalOutput")

    te = enc // P  # enc 128-row chunks per batch
    td = dec // P  # dec 128-row chunks per batch
    nk = te + td  # DVE fused ops per batch
    A = 1 + td  # ACT ops per batch (enc_row + builds)
    nblk = 512  # PSUM-bank-sized matmul block
    nh = enc // nblk

    dec_r = dec_h.ap().rearrange("(b p t) d -> b p t d", p=P, t=td)
    enc_r = enc_h.ap().rearrange("(b p t) d -> b p t d", p=P, t=te)
    out_r = out_h.ap().rearrange("(b p t) e -> b p t e", p=P, t=td)

    def enc_groups(b):
        if b == 0:
            return [(0, 2), (2, 4), (4, te)]
        if b == bpc - 1:
            return [(0, te // 2), (te // 2, te)]
        return [(0, te)]

    def dec_groups(b):
        if b == bpc - 1:
            return [(t, t + 1) for t in range(td)]
        return [(0, td)]

    with ExitStack() as ctx:

        def sb(name, shape, dt=F32):
            return ctx.enter_context(nc.sbuf_tensor(name, shape, dt))

        w_enc_b = sb("w_enc_b", [P, dim], F16)
        w_dec_b = sb("w_dec_b", [P, dim], F16)
        bias_b = sb("bias_b", [P, 1])
        ident = sb("ident", [P, P])
        ones_row = sb("ones_row", [1, P])
        enc_t = [sb(f"enc_t{i}", [P, te, dim], F16) for i in range(bpc)]
        dec_t = [sb(f"dec_t{i}", [P, td, dim], F16) for i in range(bpc)]
        scr = [sb(f"scr{i}", [P, dim], F16) for i in range(2)]
        eproj = [sb(f"eproj{i}", [P, te]) for i in range(bpc)]
        dproj = [sb(f"dproj{i}", [P, td]) for i in range(bpc)]
        enc_row = [sb(f"enc_row{i}", [1, enc]) for i in range(bpc)]
        out_t = [sb(f"out_t{i}", [P, td, enc], F16) for i in range(bpc)]
        tp_row = [
            ctx.enter_context(nc.psum_tensor(f"tp_row{i}", [1, enc], F32))
            for i in range(2)
        ]
        ebc = [
            ctx.enter_context(nc.psum_tensor(f"ebc{i}", [P, enc], F32))
            for i in range(2)
        ]

        s_we = ctx.enter_context(nc.semaphore(name="s_we"))
        s_wd = ctx.enter_context(nc.semaphore(name="s_wd"))
        s_misc = ctx.enter_context(nc.semaphore(name="s_misc"))
        s_enc = [
            [
                ctx.enter_context(nc.semaphore(name=f"s_enc{b}g{g}"))
                for g in range(len(enc_groups(b)))
            ]
            for b in range(bpc)
        ]
        s_dec = [
            [
                ctx.enter_context(nc.semaphore(name=f"s_dec{b}g{g}"))
                for g in range(len(dec_groups(b)))
            ]
            for b in range(bpc)
        ]
        s_mult = ctx.enter_context(nc.semaphore(name="s_mult"))
        s_acc = ctx.enter_context(nc.semaphore(name="s_acc"))
        s_pe = ctx.enter_context(nc.semaphore(name="s_pe"))
        s_out = ctx.enter_context(nc.semaphore(name="s_out"))

        with nc.Block(no_gpsimd_drain=True) as block:

            @block.sync
            def _(sync):
                for b in range(bpc):
                    for g, (lo, hi) in enumerate(enc_groups(b)):
                        sync.dma_start(
                            enc_t[b].ap()[:, lo:hi, :], enc_r[b][:, lo:hi, :]
                        ).then_inc(s_enc[b][g], 16)
                    for g, (lo, hi) in enumerate(dec_groups(b)):
                        sync.dma_start(
                            dec_t[b].ap()[:, lo:hi, :], dec_r[b][:, lo:hi, :]
                        ).then_inc(s_dec[b][g], 16)

            @block.gpsimd
            def _(gpsimd):
                gpsimd.dma_start(
                    w_enc_b.ap(), wenc_h.ap().to_broadcast((P, dim))
                ).then_inc(s_we, 16)
                gpsimd.dma_start(
                    w_dec_b.ap(), wdec_h.ap().to_broadcast((P, dim))
                ).then_inc(s_wd, 16)
                gpsimd.dma_start(
                    bias_b.ap(), bias_h.ap().to_broadcast((P, 1))
                ).then_inc(s_misc, 16)
                gpsimd.dma_start(ident.ap(), ident_h.ap()).then_inc(s_misc, 16)
                gpsimd.dma_start(ones_row.ap(), ones_h.ap()).then_inc(s_misc, 16)
                for b in range(bpc):
                    if b < bpc - 1:
                        gpsimd.wait_ge(s_acc, A * (b + 1))
                        nc.gpsimd.dma_start(out_r[b], out_t[b].ap()).then_inc(
                            s_out, 16
                        )
                    else:
                        for t in range(td):
                            gpsimd.wait_ge(s_acc, A * b + 1 + t + 1)
                            nc.gpsimd.dma_start(
                                out_r[b][:, t, :], out_t[b].ap()[:, t, :]
                            ).then_inc(s_out, 16)

            @block.vector
            def _(vector):
                for b in range(bpc):
                    for k in range(nk):
                        if b == 0 and k == 0:
                            vector.wait_ge(s_we, 16)
                        if b == 0 and k == te:
                            vector.wait_ge(s_wd, 16)
                        if k < te:
                            for g, (lo, hi) in enumerate(enc_groups(b)):
                                if k == lo:
                                    vector.wait_ge(s_enc[b][g], 16)
                            src, wsrc = enc_t[b].ap()[:, k, :], w_enc_b
                            tgt = eproj[b].ap()[:, k : k + 1]
                        else:
                            kd = k - te
                            for g, (lo, hi) in enumerate(dec_groups(b)):
                                if kd == lo:
                                    vector.wait_ge(s_dec[b][g], 16)
                            src, wsrc = dec_t[b].ap()[:, kd, :], w_dec_b
                            tgt = dproj[b].ap()[:, kd : kd + 1]
                        nc.vector.scalar_tensor_tensor(
                            out=scr[k % 2].ap(),
                            in0=src,
                            scalar=1.0,
                            in1=wsrc.ap(),
                            op0=mybir.AluOpType.mult,
                            op1=mybir.AluOpType.mult,
                            accum_out=tgt,
                        ).then_inc(s_mult, 1)

            @block.tensor
            def _(pe):
                for b in range(bpc):
                    if b == 0:
                        pe.wait_ge(s_misc, 48)  # ident + ones ready
                    if b >= 2:
                        # tp_row slot free once b-2's enc_row add read it
                        pe.wait_ge(s_acc, A * (b - 2) + 1)
                    pe.wait_ge(s_mult, nk * b + te)  # eproj columns ready
                    lasti = None
                    for t in range(te):
                        lasti = nc.tensor.transpose(
                            tp_row[b % 2].ap()[0:1, t * P : (t + 1) * P],
                            eproj[b].ap()[:, t : t + 1],
                            ident.ap(),
                        )
                    lasti.then_inc(s_pe, 1)
                    pe.wait_ge(s_acc, A * b + 1)  # enc_row ready
                    lasti = None
                    for h in range(nh):
                        lasti = nc.tensor.matmul(
                            ebc[b % 2].ap()[:, h * nblk : (h + 1) * nblk],
                            ones_row.ap(),
                            enc_row[b].ap()[0:1, h * nblk : (h + 1) * nblk],
                            start=True,
                            stop=True,
                        )
                    lasti.then_inc(s_pe, 1)

            @block.scalar
            def _(scalar):
                for b in range(bpc):
                    if b == 0:
                        scalar.wait_ge(s_misc, 48)
                    scalar.wait_ge(s_pe, 2 * b + 1)
                    nc.scalar.add(
                        enc_row[b].ap().rearrange("o (p t) -> o p t", p=P),
                        tp_row[b % 2].ap().rearrange("o (t p) -> o p t", p=P),
                        add=bias_b.ap()[0:1, 0:1],
                    ).then_inc(s_acc, 1)
                    scalar.wait_ge(s_pe, 2 * b + 2)
                    for t in range(td):
                        scalar.wait_ge(s_mult, nk * b + te + t + 1)
                        nc.scalar.add(
                            out_t[b].ap()[:, t, :],
                            ebc[b % 2].ap(),
                            add=dproj[b].ap()[:, t : t + 1],
                        ).then_inc(s_acc, 1)

    return nc


_NC_CACHE = {}


def _get_nc():
    if "nc" not in _NC_CACHE:
        _NC_CACHE["nc"] = _build()
    return _NC_CACHE["nc"]


_IDENT = np.eye(P, dtype=np.float32)
_ONES = np.ones((1, P), dtype=np.float32)


def _shard_inputs(decoder_states, encoder_states, mlp_weight, mlp_bias):
    decoder_states = np.asarray(decoder_states, dtype=np.float32).astype(np.float16)
    encoder_states = np.asarray(encoder_states, dtype=np.float32).astype(np.float16)
    decoder_states = np.ascontiguousarray(decoder_states)
    encoder_states = np.ascontiguousarray(encoder_states)
    mlp_weight = np.asarray(mlp_weight, dtype=np.float32).reshape(1, 2 * DIM)
    mlp_bias = np.ascontiguousarray(
        np.asarray(mlp_bias, dtype=np.float32).reshape(1, 1)
    )

    w_enc = np.ascontiguousarray(mlp_weight[:, :DIM].astype(np.float16))
    w_dec = np.ascontiguousarray(mlp_weight[:, DIM:].astype(np.float16))

    in_maps = []
    for i in range(NCORES):
        lo = i * BPC
        in_maps.append(
            {
                "dec_in": decoder_states[lo : lo + BPC].reshape(BPC * DEC, DIM),
                "enc_in": encoder_states[lo : lo + BPC].reshape(BPC * ENC, DIM),
                "w_enc": w_enc,
                "w_dec": w_dec,
                "bias": mlp_bias,
                "ident_in": _IDENT,
                "ones_in": _ONES,
            }
        )
    return in_maps


def _gather(res):
    shards = [
        r["out"].astype(np.float32).reshape(BPC, DEC, ENC) for r in res.results
    ]
    return np.concatenate(shards, axis=0)


def kernel(decoder_states, encoder_states, step, mlp_weight, mlp_bias, **_ignored):
    in_maps = _shard_inputs(decoder_states, encoder_states, mlp_weight, mlp_bias)
    res = run_bass_kernel_spmd(_get_nc(), in_maps, core_ids=list(range(NCORES)))
    return _gather(res)
